# revision 2
# baseline (speedup 1.0000x reference)
"""Trainium2 Bass kernel for int4-grouped-quantized linear (GPTQ-style), v6.

out[8192, 11008] = x[8192, 4096] @ dequant(qweight, qzeros, scales)

Column-parallel over N across 8 NeuronCores. vs v3/v5:
- x loads via DMA X-bar transpose on the Sync ring (PE does zero transposes).
- All dequant DMAs ride the Scalar HWDGE ring, front-loaded: 4 big qweight
  quarter-loads + full-width z/s partition-broadcasts (HWDGE broadcast
  measured ~0.7us vs ~6.5us serialized on SWDGE).
- int4 unpack is 2 DVE ops per 4-block batch (TT shift with a stride-0
  repeat-4 read against a [0,4,8,12] vector, then &0xF), i16 in/out to
  satisfy the no-cast bitVec rule; ACT copies i16->f16.
- Ramp: chunk 0 runs 4 PSUM groups of width 1024 (8 banks, k-outer
  interleaved) so the PE absorbs ~55us of work while dequant streams;
  the 352-wide residual of those 4 row-blocks runs right after.
"""

import sys

sys.path.insert(0, "/opt/trn_rl_repo")

from contextlib import ExitStack

import numpy as np

import concourse.bass as bass
from concourse import bacc
import concourse.tile as tile
from concourse import mybir
from concourse.bass_utils import run_bass_kernel_spmd

AOT = mybir.AluOpType
F16, I16, I32, F32 = (
    mybir.dt.float16,
    mybir.dt.int16,
    mybir.dt.int32,
    mybir.dt.float32,
)

T, K, N = 8192, 4096, 11008
NCORES = 8
NS = N // NCORES  # 1376 out cols per core
CS = NS // 8  # 172 packed int32 cols per core
G = 32  # quant groups (group size 128 == one k-block)
KB = K // 128  # 32 k-blocks
Q = 4  # k-blocks per dequant batch
NQ = KB // Q  # 8 dequant batches
QW_QUARTER = 8  # k-blocks per qweight load (2 batches)
CH = 512  # x rows per transposed chunk
NCH = T // CH  # 16 chunks
RBC = CH // 128  # 4 row-blocks per chunk
SEGS = [(0, 512), (512, 512), (1024, 352)]  # N segments (PSUM bank sized)


def _body(ctx, tc, xd, qwd, qzd, scd, outd, zscr):
    nc = tc.nc
    cpool = ctx.enter_context(tc.tile_pool(name="const", bufs=1))
    qpool = ctx.enter_context(tc.tile_pool(name="qwp", bufs=4))
    stpool = ctx.enter_context(tc.tile_pool(name="stage", bufs=2))
    wpool = ctx.enter_context(tc.tile_pool(name="w", bufs=NQ))
    bcpool = ctx.enter_context(tc.tile_pool(name="bc", bufs=1))
    xtpool = ctx.enter_context(tc.tile_pool(name="xt", bufs=42))
    opool = ctx.enter_context(tc.tile_pool(name="o", bufs=4))

    # ---- dequant DMA front-load: qz, qweight quarters, shift vector ----
    qz_t = cpool.tile([G, CS], I32)
    nc.scalar.dma_start(qz_t[:], qzd)
    qw_tiles = []
    for h in range(4):
        qw_t = qpool.tile([128, QW_QUARTER * CS], I32, name="qw", tag="qw")
        nc.scalar.dma_start(
            qw_t[:].rearrange("p (b c) -> p b c", b=QW_QUARTER),
            qwd[h * QW_QUARTER * 128 : (h + 1) * QW_QUARTER * 128, :].rearrange(
                "(b p) c -> p b c", p=128
            ),
        )
        qw_tiles.append(qw_t)

    sv = cpool.tile([128, 4], I16)
    for r in range(4):
        nc.vector.memset(sv[:, r : r + 1], 4 * r)

    def transpose_chunk(c):
        r0 = c * CH
        xts = []
        for b in range(KB):
            xt = xtpool.tile([128, CH], F16, tag="xt")
            nc.sync.dma_start(
                xt[:], xd[r0 : r0 + CH, b * 128 : (b + 1) * 128], transpose=True
            )
            xts.append(xt)
        return xts

    xts0 = transpose_chunk(0)

    # ---- zero-points: qz -> z [G, NS] f16, park in DRAM for broadcasts ----
    z_stage = cpool.tile([G, NS], I32)
    for j in range(8):
        nc.vector.tensor_scalar(
            z_stage[:, j::8], qz_t[:], 4 * j, 0xF,
            AOT.logical_shift_right, AOT.bitwise_and,
        )
    z_t = cpool.tile([G, NS], F16)
    nc.vector.tensor_copy(z_t[:], z_stage[:])
    nc.scalar.dma_start(zscr, z_t[:])

    # ---- dequantize W, 4 k-blocks per batch, TT-shift unpack ----
    w_tiles = []  # [128, Q*NS] per batch; block b -> w_tiles[b//Q][:, (b%Q)*NS:...]
    for q in range(NQ):
        qsrc = qw_tiles[q // 2][:].bitcast(I16)  # [128, 2*QW_QUARTER*CS]
        lo = (q % 2) * Q * 2 * CS
        seg16 = qsrc[:, lo : lo + Q * 2 * CS]  # this batch's i16 cols [128, 1376]
        rep4 = seg16.unsqueeze(2).broadcast_to([128, Q * 2 * CS, 4])
        svb = sv[:].unsqueeze(1).broadcast_to([128, Q * 2 * CS, 4])
        w_stage = stpool.tile([128, Q * NS], I16)
        st3 = w_stage[:].rearrange("p (m r) -> p m r", r=4)
        nc.vector.tensor_tensor(st3, rep4, svb, AOT.logical_shift_right)
        nc.vector.tensor_scalar(w_stage[:], w_stage[:], 0xF, None, AOT.bitwise_and)
        w_t = wpool.tile([128, Q * NS], F16)
        nc.scalar.copy(w_t[:], w_stage[:])
        H = Q * NS // 2
        for hh in range(2):
            hs = slice(hh * H, (hh + 1) * H)
            gg = Q * q + 2 * hh
            z_bc = bcpool.tile([128, H], F16, name="zbc", tag="zbc")
            nc.sync.dma_start(
                z_bc[:],
                zscr[gg : gg + 2, :]
                .rearrange("(o g) n -> o (g n)", o=1)
                .partition_broadcast(128),
            )
            s_bc = bcpool.tile([128, H], F16, name="sbc", tag="sbc")
            nc.sync.dma_start(
                s_bc[:],
                scd[gg : gg + 2, :]
                .rearrange("(o g) n -> o (g n)", o=1)
                .partition_broadcast(128),
            )
            nc.vector.tensor_tensor(w_t[:, hs], w_t[:, hs], z_bc[:], AOT.subtract)
            nc.vector.tensor_tensor(w_t[:, hs], w_t[:, hs], s_bc[:], AOT.mult)
        w_tiles.append(w_t)

    def wslice(b, off, sz):
        base = (b % Q) * NS
        return w_tiles[b // Q][:, base + off : base + off + sz]

    def evac(ps_ap, ob, cols, rr, last):
        nc.scalar.copy(ob[:, cols], ps_ap)
        if last:
            nc.scalar.dma_start(outd[rr : rr + 128, :], ob[:])

    pspool = ctx.enter_context(tc.tile_pool(name="ps", bufs=8, space="PSUM"))

    def bank(n):
        return pspool.tile([128, 512], F32, name="psb", tag="psb")

    # ---- chunk 0: ramp. 4 PSUM groups of width 1024 (8 banks), k-outer. ----
    rps = [[bank(0), bank(1)] for _ in range(RBC)]
    for b in range(KB):
        for rb in range(RBC):
            lhsT = xts0[b][:, rb * 128 : (rb + 1) * 128]
            for i in range(2):
                nc.tensor.matmul(
                    rps[rb][i][:],
                    lhsT,
                    wslice(b, i * 512, 512),
                    start=(b == 0),
                    stop=(b == KB - 1),
                )
    obs0 = []
    for rb in range(RBC):
        ob = opool.tile([128, NS], F16, name="ob", tag="ob")
        for i in range(2):
            evac(rps[rb][i][:], ob, slice(i * 512, (i + 1) * 512), None, last=False)
        obs0.append(ob)

    # residual 352-wide strip of chunk 0's 4 row-blocks (4 banks, k-outer)
    res_ps = [bank(0) for _ in range(RBC)]
    for b in range(KB):
        for rb in range(RBC):
            nc.tensor.matmul(
                res_ps[rb][:, 0:352],
                xts0[b][:, rb * 128 : (rb + 1) * 128],
                wslice(b, 1024, 352),
                start=(b == 0),
                stop=(b == KB - 1),
            )
    for rb in range(RBC):
        evac(res_ps[rb][:, 0:352], obs0[rb], slice(1024, NS), rb * 128, last=True)

    # ---- chunks 1..15: serial row-blocks, 3 banks per group ----
    for c in range(1, NCH):
        r0 = c * CH
        xts = transpose_chunk(c)
        for rb in range(RBC):
            ts = [bank(0) for _ in range(3)]
            for b in range(KB):
                lhsT = xts[b][:, rb * 128 : (rb + 1) * 128]
                for i, (off, sz) in enumerate(SEGS):
                    nc.tensor.matmul(
                        ts[i][:, 0:sz],
                        lhsT,
                        wslice(b, off, sz),
                        start=(b == 0),
                        stop=(b == KB - 1),
                    )
            ob = opool.tile([128, NS], F16, name="ob", tag="ob")
            for i, (off, sz) in enumerate(SEGS):
                evac(ts[i][:, 0:sz], ob, slice(off, off + sz), None, last=False)
            nc.scalar.dma_start(outd[r0 + rb * 128 : r0 + rb * 128 + 128, :], ob[:])


def build_kernel():
    nc = bacc.Bacc("TRN2", target_bir_lowering=False, debug=False)
    xd = nc.dram_tensor("x", [T, K], F16, kind="ExternalInput").ap()
    qwd = nc.dram_tensor("qw", [K, CS], I32, kind="ExternalInput").ap()
    qzd = nc.dram_tensor("qz", [G, CS], I32, kind="ExternalInput").ap()
    scd = nc.dram_tensor("sc", [G, NS], F16, kind="ExternalInput").ap()
    outd = nc.dram_tensor("out", [T, NS], F16, kind="ExternalOutput").ap()
    zscr = nc.dram_tensor("z_scratch", [G, NS], F16, kind="Internal").ap()
    with tile.TileContext(nc) as tc, ExitStack() as ctx:
        _body(ctx, tc, xd, qwd, qzd, scd, outd, zscr)
    nc.compile()
    return nc


_NC = None


def _get_nc():
    global _NC
    if _NC is None:
        _NC = build_kernel()
    return _NC


def make_in_maps(x, qweight, qzeros, scales):
    x = np.asarray(x, dtype=np.float16)
    qweight = np.asarray(qweight, dtype=np.int32)
    qzeros = np.asarray(qzeros, dtype=np.int32)
    scales = np.asarray(scales, dtype=np.float16)
    in_maps = []
    for c in range(NCORES):
        in_maps.append(
            {
                "x": x,
                "qw": np.ascontiguousarray(qweight[:, c * CS : (c + 1) * CS]),
                "qz": np.ascontiguousarray(qzeros[:, c * CS : (c + 1) * CS]),
                "sc": np.ascontiguousarray(scales[:, c * NS : (c + 1) * NS]),
            }
        )
    return in_maps


def run(in_maps, **kwargs):
    return run_bass_kernel_spmd(
        _get_nc(), in_maps, core_ids=list(range(NCORES)), **kwargs
    )


def kernel(x, qweight, qzeros, scales):
    res = run(make_in_maps(x, qweight, qzeros, scales))
    outs = [res.results[c]["out"] for c in range(NCORES)]
    return np.concatenate(outs, axis=1)
